# revision 27
# baseline (speedup 1.0000x reference)
"""Trainium2 Bass kernel for BatchedCrossColumnAttentionCompressed.

Strategy (sharding_hint): shard leading N (column) axis across the 8 cores.
Each core: LN -> (folded) compress projections -> quantize -> AllReduce of the
small [TOK, 2R] compressed tensor -> decompress -> causal SDPA -> out proj.

Host-side algebraic folding (exact linear-map collapses):
  - LN affine (w,b) folded into projection weights (biases are zero for the
    actual inputs -> bias paths elided at build time).
  - k/v D->D projection collapsed into the D->R compression: w_kc = k_comp @ w_k_eff.
  - col_mask folded into w_kc/w_vc; 1/n_active folded into decompress weights.
  - 1/sqrt(HD) folded into q projection; k_dec folded into q projection.
  - v_dec folded into w_o: wvo_h = (W_oh @ Vd_h)^T  [R, D] -- the attention
    output projection consumes the compressed context directly (no decompress
    stage on device).
Softmax: scores are tiny (|s| << 1), so max-subtraction is skipped (exact same
math as reference up to fp rounding). exp computed on ACT over 1024-wide PSUM
windows; normalization deferred to phase F (per-head scaling pre-sum).
quant_ste round() implemented with the fp32 magic-constant RNE trick.

Scores matmuls are K=64 row-tiles: kavg^T is duplicated into partitions
64-127 and the two heads of a pair run concurrently on PE row-groups
(0,0)/(64,0). q projection emits head pairs stacked [128, tok] directly.
"""

import numpy as np
import ml_dtypes

N, B, T, D = 8, 4, 1024, 512
H = 4
HD = D // H           # 128
R = 64
R2 = 2 * R            # 128
EPS = 1e-5
TOK = B * T           # 4096
NTI = TOK // 128      # 32 token chunks
KD = D // 128         # 4 contraction chunks
NQ = T // 128         # 8 q-chunks per batch row
MAGIC = 12582912.0    # 1.5 * 2^23 -> round-to-nearest-even trick

_STATE = {}

# attn^T strip packing: strip ki holds q in [ki*128, T). Strips are
# reordered into 1024-wide windows ([s0],[s1,s7],[s2,s6],[s3,s5],[s4]) so
# every 512-aligned exp window is filled and chunks never split banks.
STRIPW = [T - 128 * ki for ki in range(NQ)]
_order = [0, 1, 7, 2, 6, 3, 5, 4]
OFFS = [0] * NQ
_off = 0
for _ki in _order:
    OFFS[_ki] = _off
    _off += STRIPW[_ki]
TOTW = sum(STRIPW)                    # 4608
WINW = 1024                            # exp window width (2 PSUM banks)
NWIN = (TOTW + WINW - 1) // WINW       # 5

# per-window list of (ki, g0, g1): global attnT cols [g0,g1) from strip ki,
# split at 512 (PSUM bank) boundaries
WIN_CHUNKS = []
for w in range(NWIN):
    lo, hi = w * WINW, min((w + 1) * WINW, TOTW)
    chunks = []
    for ki in range(NQ):
        s0, s1 = OFFS[ki], OFFS[ki] + STRIPW[ki]
        c0, c1 = max(s0, lo), min(s1, hi)
        g = c0
        while g < c1:
            nxt = min(c1, (g // 512 + 1) * 512)
            chunks.append((ki, g, nxt))
            g = nxt
    WIN_CHUNKS.append((lo, hi, chunks))


def _build_program(with_kv_bias, with_q_bias):
    from concourse import bacc
    import concourse.bass as bass
    import concourse.tile as tile
    import concourse.mybir as mybir

    f32 = mybir.dt.float32
    bf16 = mybir.dt.bfloat16
    AF = mybir.ActivationFunctionType
    ALU = mybir.AluOpType
    AX = mybir.AxisListType

    nc = bacc.Bacc("TRN2", target_bir_lowering=False, debug=False, num_devices=N)

    x_d = nc.dram_tensor("x", [TOK, D], f32, kind="ExternalInput").ap()
    wkv_d = nc.dram_tensor("wkv", [D, R2], bf16, kind="ExternalInput").ap()
    wqk_d = nc.dram_tensor("wqk", [D, H * R], bf16, kind="ExternalInput").ap()
    wvo_d = nc.dram_tensor("wvo", [R2, 2 * D], bf16, kind="ExternalInput").ap()
    ident_d = nc.dram_tensor("ident", [128, 128], bf16, kind="ExternalInput").ap()
    cmask_d = nc.dram_tensor("cmask", [128, 128], bf16,
                             kind="ExternalInput").ap()
    if with_kv_bias:
        bkv_d = nc.dram_tensor("bkv", [1, R2], bf16, kind="ExternalInput").ap()
    if with_q_bias:
        bq_d = nc.dram_tensor("bq", [1, H * R], bf16, kind="ExternalInput").ap()
    out_d = nc.dram_tensor("out", [TOK, D], f32, kind="ExternalOutput").ap()

    with tile.TileContext(nc) as tc:
        with (
            tc.tile_pool(name="consts", bufs=1) as consts,
            tc.tile_pool(name="big", bufs=1) as big,
            tc.tile_pool(name="work", bufs=3) as work,
            tc.tile_pool(name="work2", bufs=2) as work2,
            tc.tile_pool(name="small", bufs=24) as small,
            # PSUM: psS 2x4KB (scores windows) + psAv 2x2KB + psF 2x2KB = 16KB
            tc.tile_pool(name="psS", bufs=2, space="PSUM") as psS,
            tc.tile_pool(name="psAv", bufs=2, space="PSUM") as psAv,
            tc.tile_pool(name="psF", bufs=2, space="PSUM") as psF,
            tc.tile_pool(name="dram", bufs=1, space="DRAM") as dpool,
        ):
            # AR chunk sizes in 128-token tiles: first batch row split in
            # half so the first collective (whose staging is BW-bound) lands
            # as early as possible
            SPLITS = [8, 8, 8, 8]
            CUM = [0]
            for s in SPLITS:
                CUM.append(CUM[-1] + s)
            ar_ins = [dpool.tile([SPLITS[q] * 128, R2], bf16,
                                 name=f"ar_in{q}") for q in range(len(SPLITS))]
            ar_outs = [dpool.tile([SPLITS[q] * 128, R2], bf16,
                                  name=f"ar_out{q}", addr_space="Shared")
                       for q in range(len(SPLITS))]

            def ar_buf(ti):
                q = 0
                while CUM[q + 1] <= ti:
                    q += 1
                return q, ti - CUM[q]

            # ---- constants ----
            ident = consts.tile([128, 128], bf16, name="ident")
            nc.sync.dma_start(out=ident, in_=ident_d)
            cmask = consts.tile([128, 128], bf16, name="cmask")
            nc.sync.dma_start(out=cmask, in_=cmask_d)
            wkv_s = []
            for kd in range(KD):
                wkvt = consts.tile([128, R2], bf16, name=f"wkv{kd}")
                nc.sync.dma_start(out=wkvt, in_=wkv_d[kd * 128:(kd + 1) * 128, :])
                wkv_s.append(wkvt)
            wqk_s = []
            for kd in range(KD):
                wqkt = consts.tile([128, H * R], bf16, name=f"wqk{kd}")
                nc.sync.dma_start(out=wqkt,
                                  in_=wqk_d[kd * 128:(kd + 1) * 128, :])
                wqk_s.append(wqkt)
            wvo_sb = consts.tile([R2, 2 * D], bf16, name="wvo_sb")
            nc.sync.dma_start(out=wvo_sb, in_=wvo_d)
            ones_col = consts.tile([128, 1], bf16, name="ones_col")
            nc.vector.memset(ones_col, 1.0)
            eps_t = consts.tile([128, 1], f32, name="eps_t")
            nc.vector.memset(eps_t, EPS)
            ones_row = consts.tile([1, 512], bf16, name="ones_row")
            nc.vector.memset(ones_row, 1.0)
            if with_kv_bias:
                bkv_s = consts.tile([1, R2], bf16, name="bkv_s")
                nc.sync.dma_start(out=bkv_s, in_=bkv_d)
            if with_q_bias:
                bq_s = consts.tile([1, H * R], bf16, name="bq_s")
                nc.sync.dma_start(out=bq_s, in_=bq_d)

            # ---- persistent big tensors ----
            nt = big.tile([128, KD, TOK], bf16, name="nt")
            # q projected into compressed space, head-PAIRS stacked on
            # partitions: rows 0-63 = head 2p, 64-127 = head 2p+1
            qdecT = big.tile([128, 2, TOK], bf16, name="qdecT")
            # kavg^T duplicated into both partition halves (row-tiling)
            kavgT = big.tile([128, TOK], bf16, name="kavgT")
            # v_avg natural chunks with a ones column at index R (Z trick)
            vext = big.tile([128, NTI, R + 1], bf16, name="vext")
            # compressed context per batch (double-buffered on b parity):
            # rows 0-63 ctx, row 64 = Z
            outcB = big.tile([R + 1, H, 2, T], bf16, name="outcB")
            # odd heads' ctx shifted to partitions 64-127 so phase-F matmuls
            # alternate PE row-halves (complementary row-groups overlap)
            outcHi = big.tile([128, 2, 2, T], bf16, name="outcHi")
            recipsAll = big.tile([128, B * H * NQ], f32, name="recipsAll")

            # ================= Phase A: LN + transpose + compress + quant ====
            def emit_A(t0, t1):
                for ti in range(t0, t1):
                    tsl = slice(ti * 128, (ti + 1) * 128)
                    xt = work.tile([128, D], f32, tag="xt", bufs=8, name="xt")
                    nc.sync.dma_start(out=xt, in_=x_d[tsl, :])
                    stats = small.tile([128, 6], f32, name="stats")
                    nc.vector.bn_stats(out=stats, in_=xt)
                    mv = small.tile([128, 2], f32, name="mv")
                    nc.vector.bn_aggr(out=mv, in_=stats)
                    std = small.tile([128, 1], f32, name="std")
                    nc.scalar.activation(out=std, in_=mv[:, 1:2], func=AF.Sqrt,
                                         bias=eps_t, scale=1.0)
                    rstd = small.tile([128, 1], f32, name="rstd")
                    nc.vector.reciprocal(out=rstd, in_=std)
                    # nbias = -mean*rstd ; normed = x*rstd + nbias  (on ACT)
                    nbias = small.tile([128, 1], f32, name="nbias")
                    nc.vector.tensor_scalar(out=nbias, in0=mv[:, 0:1],
                                            scalar1=rstd, scalar2=-1.0,
                                            op0=ALU.mult, op1=ALU.mult)
                    nrm = work.tile([128, D], bf16, tag="nrm", bufs=8, name="nrm")
                    nc.scalar.activation(out=nrm, in_=xt, func=AF.Identity,
                                         bias=nbias, scale=rstd)
                    pst = psS.tile([128, KD * 128], bf16, tag="psS", name="pst")
                    for kd in range(KD):
                        nc.tensor.transpose(pst[:, kd * 128:(kd + 1) * 128],
                                            nrm[:, kd * 128:(kd + 1) * 128],
                                            ident)
                    nc.vector.tensor_copy(
                        out=nt[:, :, tsl],
                        in_=pst.rearrange("p (g c) -> p g c", g=KD))
                    pskv = psAv.tile([128, R2], f32, tag="psAv", name="pskv")
                    for kd in range(KD):
                        nc.tensor.matmul(pskv, lhsT=nt[:, kd, tsl], rhs=wkv_s[kd],
                                         start=(kd == 0),
                                         stop=(kd == KD - 1 and not with_kv_bias))
                    if with_kv_bias:
                        nc.tensor.matmul(pskv, lhsT=bkv_s, rhs=ones_row[:, 0:128],
                                         start=False, stop=True)
                    absm = small.tile([128, 2], f32, name="absm")
                    nc.vector.tensor_reduce(
                        out=absm,
                        in_=pskv.rearrange("p (g r) -> p g r", g=2),
                        axis=AX.X, op=ALU.max, apply_absolute_value=True)
                    # inv_s = max(absm,1e-8)/127 ; sc = 1/inv_s ; mb = -MAGIC*inv_s
                    inv_s = small.tile([128, 2], f32, name="inv_s")
                    nc.vector.tensor_scalar(out=inv_s, in0=absm, scalar1=1e-8,
                                            scalar2=1.0 / 127.0, op0=ALU.max,
                                            op1=ALU.mult)
                    sc = small.tile([128, 2], f32, name="sc")
                    nc.vector.reciprocal(out=sc, in_=inv_s)
                    mb = small.tile([128, 2], f32, name="mb")
                    nc.vector.tensor_scalar_mul(out=mb, in0=inv_s, scalar1=-MAGIC)
                    arq = work.tile([128, R2], bf16, tag="arq", bufs=8, name="arq")
                    tmpq = work.tile([128, R2], f32, tag="tmpq", bufs=8, name="tmpq")
                    for half in range(2):
                        sl = slice(half * R, (half + 1) * R)
                        hh = slice(half, half + 1)
                        # y = x*sc + MAGIC  (rounds to int in fp32 mantissa)
                        if half == 0:
                            nc.vector.tensor_scalar(out=tmpq[:, sl],
                                                    in0=pskv[:, sl],
                                                    scalar1=sc[:, hh],
                                                    scalar2=MAGIC,
                                                    op0=ALU.mult, op1=ALU.add)
                        else:
                            nc.scalar.activation(out=tmpq[:, sl],
                                                 in_=pskv[:, sl],
                                                 func=AF.Copy, bias=MAGIC,
                                                 scale=sc[:, hh])
                        # q = (y - MAGIC)*inv_s = y*inv_s + mb
                        nc.scalar.activation(out=arq[:, sl], in_=tmpq[:, sl],
                                             func=AF.Identity, bias=mb[:, hh],
                                             scale=inv_s[:, hh])
                    q_, r_ = ar_buf(ti)
                    nc.sync.dma_start(
                        out=ar_ins[q_][r_ * 128:(r_ + 1) * 128, :],
                        in_=arq)

            # ================= Phase B: AllReduce (quartered pipeline) ======
            def emit_AR(which):
                nc.gpsimd.collective_compute(
                    "AllReduce",
                    ALU.add,
                    replica_groups=[list(range(N))],
                    ins=[ar_ins[which].opt()],
                    outs=[ar_outs[which].opt()],
                )

            # ================= Phase C: q^T pair projection (overlaps AR) ===
            def emit_C(n0, n1):
                for p in range(2):
                    for nch in range(n0, n1):
                        csl = slice(nch * 512, (nch + 1) * 512)
                        psq = psF.tile([128, 512], f32, tag="psF", name="psq")
                        for kd in range(KD):
                            nc.tensor.matmul(
                                psq,
                                lhsT=wqk_s[kd][:, p * 128:(p + 1) * 128],
                                rhs=nt[:, kd, csl],
                                start=(kd == 0),
                                stop=(kd == KD - 1 and not with_q_bias),
                            )
                        if with_q_bias:
                            nc.tensor.matmul(psq,
                                             lhsT=bq_s[:, p * 128:(p + 1) * 128],
                                             rhs=ones_row, start=False, stop=True)
                        nc.vector.tensor_copy(out=qdecT[:, p, csl], in_=psq)

            # ================= Phase D: k_avg^T dup transpose + v_ext =======
            def emit_D(t0, t1):
                for ti in range(t0, t1):
                    tsl = slice(ti * 128, (ti + 1) * 128)
                    avgN = work.tile([128, R2], bf16, tag="avgN", bufs=6, name="avgN")
                    q_, r_ = ar_buf(ti)
                    src = ar_outs[q_][r_ * 128:(r_ + 1) * 128, :]
                    nc.sync.dma_start(out=avgN, in_=src)
                    psK = psAv.tile([128, 128], f32, tag="psAv", name="psK")
                    nc.tensor.matmul(psK[0:64, :], lhsT=avgN[:, 0:R], rhs=ident,
                                     start=True, stop=True)
                    nc.tensor.matmul(psK[64:128, :], lhsT=avgN[:, 0:R], rhs=ident,
                                     start=True, stop=True)
                    nc.vector.tensor_copy(out=kavgT[:, tsl], in_=psK)
                    nc.gpsimd.tensor_copy(out=vext[:, ti, 0:R],
                                          in_=avgN[:, R:R2])

            # ================= Phase E: causal SDPA (compressed, rank-R) ====
            # scoresT strips for the two heads of a pair computed CONCURRENTLY
            # as K=64 row-tiles (kavgT dup at partitions 64-127). exp over
            # 1024-wide PSUM windows -> attn^T strips in SBUF. AV contracts
            # attn^T against [v_avg | ones] chunks: psc row R gives Z, rows
            # 0..R-1 the compressed context (-> outcB, consumed by phase F).
            GQ = 4  # q-chunks per AV group (512-wide matmuls)
            def emit_E(b, p):
                base = b * T
                if True:
                    attnP = work2.tile([128, 2, TOTW], bf16, name="attnP")
                    attnE = attnP[:, 0, :]
                    attnO = attnP[:, 1, :]
                    for w in range(NWIN):
                        lo, hi, chunks = WIN_CHUNKS[w]
                        tE = psS.tile([128, WINW], f32, tag="psS", name="tE")
                        tO = psS.tile([128, WINW], f32, tag="psS", name="tO")
                        for (ki, g0, g1) in chunks:
                            q0 = base + ki * 128 + (g0 - OFFS[ki])
                            q1 = q0 + (g1 - g0)
                            kb = slice(base + ki * 128, base + (ki + 1) * 128)
                            nc.tensor.matmul(
                                tE[:, g0 - lo:g1 - lo],
                                lhsT=kavgT[0:64, kb],
                                rhs=qdecT[0:64, p, q0:q1],
                                start=True, stop=True)
                            nc.tensor.matmul(
                                tO[:, g0 - lo:g1 - lo],
                                lhsT=kavgT[64:128, kb],
                                rhs=qdecT[64:128, p, q0:q1],
                                start=True, stop=True)
                        nc.scalar.activation(out=attnE[:, lo:hi],
                                             in_=tE[:, 0:hi - lo], func=AF.Exp)
                        nc.scalar.activation(out=attnO[:, lo:hi],
                                             in_=tO[:, 0:hi - lo], func=AF.Exp)
                        # causal zeroing of diag blocks living in this window
                        for ki in range(NQ):
                            if lo <= OFFS[ki] < hi:
                                dsl = slice(OFFS[ki], OFFS[ki] + 128)
                                nc.vector.tensor_tensor(
                                    out=attnE[:, dsl], in0=attnE[:, dsl],
                                    in1=cmask, op=ALU.mult)
                                nc.vector.tensor_tensor(
                                    out=attnO[:, dsl], in0=attnO[:, dsl],
                                    in1=cmask, op=ALU.mult)
                    for h, attnTs in ((2 * p, attnE), (2 * p + 1, attnO)):
                        for g in range(NQ // GQ):
                            q0 = g * GQ          # first q-chunk of group
                            gw = GQ * 128        # 512
                            gsl = slice(q0 * 128, (q0 + GQ) * 128)
                            psc = psAv.tile([R + 1, gw], f32, tag="psAv", name="psc")
                            for ki in range(q0 + GQ):
                                lo2 = max(ki, q0)
                                nc.tensor.matmul(
                                    psc[:, (lo2 - q0) * 128:gw],
                                    lhsT=vext[:, b * NQ + ki, :],
                                    rhs=attnTs[:, OFFS[ki] + (lo2 - ki) * 128:
                                               OFFS[ki] +
                                               (q0 + GQ - ki) * 128],
                                    start=(ki == 0), stop=(ki == q0 + GQ - 1),
                                    skip_group_check=True)
                            if g % 2 == 0:
                                nc.vector.tensor_copy(
                                    out=outcB[:, h, b % 2, gsl], in_=psc)
                            else:
                                nc.scalar.copy(
                                    out=outcB[:, h, b % 2, gsl], in_=psc)
                        # Z row -> psz columns (outer-product transposes)
                        psz = psAv.tile([128, NQ], f32, tag="psAv", name="psz")
                        for qi in range(NQ):
                            nc.tensor.matmul(
                                psz[:, qi:qi + 1],
                                lhsT=outcB[R:R + 1, h, b % 2,
                                           qi * 128:(qi + 1) * 128],
                                rhs=ones_col[R:R + 1, 0:1],
                                start=True, stop=True)
                        idx0 = (b * H + h) * NQ
                        zcol = small.tile([128, NQ], f32, name="zcol")
                        nc.vector.tensor_copy(out=zcol, in_=psz)
                        nc.vector.reciprocal(
                            out=recipsAll[:, idx0:idx0 + NQ], in_=zcol)
                        if h % 2 == 1:
                            nc.sync.dma_start(
                                out=outcHi[64:128, h // 2, b % 2, :],
                                in_=outcB[0:R, h, b % 2, :])
            # ================= Phase F: out proj + residual + normalize =====
            # of[tok, D] = x + sum_h (1/Z_h) * (outc_h^T @ wvo_h)   (K = R)
            # Emitted in two passes (one per head pair) so pass 0 of batch b
            # overlaps the second E pair of the same batch.
            ofs = {}

            def emit_F(b, p):
                for qi in range(NQ):
                    ti = b * NQ + qi
                    tsl = slice(ti * 128, (ti + 1) * 128)
                    qsl = slice(qi * 128, (qi + 1) * 128)
                    if p == 0:
                        xt2 = work.tile([128, D], f32, tag="xt", bufs=8,
                                        name="xt")
                        nc.sync.dma_start(out=xt2, in_=x_d[tsl, :])
                        of = work.tile([128, D], f32, tag="of", bufs=10,
                                       name="of")
                        ofs[qi] = (of, xt2)
                    else:
                        of, xt2 = ofs[qi]
                    psoE = psF.tile([128, 512], f32, tag="psF", name="psoE")
                    psoO = psF.tile([128, 512], f32, tag="psF", name="psoO")
                    nc.tensor.matmul(
                        psoE,
                        lhsT=outcB[0:R, 2 * p, b % 2, qsl],
                        rhs=wvo_sb[0:R, p * D:(p + 1) * D],
                        start=True, stop=True)
                    nc.tensor.matmul(
                        psoO,
                        lhsT=outcHi[64:128, p, b % 2, qsl],
                        rhs=wvo_sb[64:128, p * D:(p + 1) * D],
                        start=True, stop=True)
                    r0 = (b * H + 2 * p) * NQ + qi
                    r1 = (b * H + 2 * p + 1) * NQ + qi
                    nc.vector.scalar_tensor_tensor(
                        out=of, in0=psoE,
                        scalar=recipsAll[:, r0:r0 + 1],
                        in1=(xt2 if p == 0 else of),
                        op0=ALU.mult, op1=ALU.add)
                    nc.vector.scalar_tensor_tensor(
                        out=of, in0=psoO,
                        scalar=recipsAll[:, r1:r1 + 1],
                        in1=of, op0=ALU.mult, op1=ALU.add)
                    if p == 1:
                        nc.sync.dma_start(out=out_d[tsl, :], in_=of)

            # ---- pipelined emission order ----
            nc.vector.memset(vext[:, :, R:R + 1], 1.0)
            with nc.named_scope("A1"):
                emit_A(0, 8)
            with nc.named_scope("AR0"):
                emit_AR(0)
            with nc.named_scope("C1"):
                emit_C(0, 2)
            with nc.named_scope("A2"):
                emit_A(8, 16)
            with nc.named_scope("AR1"):
                emit_AR(1)
            with nc.named_scope("C2"):
                emit_C(2, 4)
            with nc.named_scope("A3"):
                emit_A(16, 24)
            with nc.named_scope("AR2"):
                emit_AR(2)
            with nc.named_scope("C3"):
                emit_C(4, 6)
            with nc.named_scope("A4"):
                emit_A(24, 32)
            with nc.named_scope("AR3"):
                emit_AR(3)
            with nc.named_scope("C4"):
                emit_C(6, 8)
            with nc.named_scope("D1"):
                emit_D(0, 8)
            with nc.named_scope("E0a"):
                emit_E(0, 0)
            with nc.named_scope("E0b"):
                emit_E(0, 1)
            with nc.named_scope("F0a"):
                emit_F(0, 0)
            with nc.named_scope("D2"):
                emit_D(8, 16)
            with nc.named_scope("F0b"):
                emit_F(0, 1)
            with nc.named_scope("E1a"):
                emit_E(1, 0)
            with nc.named_scope("F1a"):
                emit_F(1, 0)
            with nc.named_scope("E1b"):
                emit_E(1, 1)
            with nc.named_scope("D3"):
                emit_D(16, 24)
            with nc.named_scope("F1b"):
                emit_F(1, 1)
            with nc.named_scope("E2a"):
                emit_E(2, 0)
            with nc.named_scope("F2a"):
                emit_F(2, 0)
            with nc.named_scope("E2b"):
                emit_E(2, 1)
            with nc.named_scope("D4"):
                emit_D(24, 32)
            with nc.named_scope("F2b"):
                emit_F(2, 1)
            with nc.named_scope("E3a"):
                emit_E(3, 0)
            with nc.named_scope("F3a"):
                emit_F(3, 0)
            with nc.named_scope("E3b"):
                emit_E(3, 1)
            with nc.named_scope("F3b"):
                emit_F(3, 1)

    nc.compile()
    return nc


def _prepare(inputs):
    bf = ml_dtypes.bfloat16
    x = np.ascontiguousarray(np.asarray(inputs["col_states"], np.float32))
    mask_f = np.asarray(inputs["col_mask"]).astype(np.float32)
    n_active = max(float(mask_f.sum()), 1.0)

    lw_kv = np.asarray(inputs["ln_kv_w"], np.float32).reshape(N, D)
    lb_kv = np.asarray(inputs["ln_kv_b"], np.float32).reshape(N, D)
    lw_q = np.asarray(inputs["ln_q_w"], np.float32).reshape(N, D)
    lb_q = np.asarray(inputs["ln_q_b"], np.float32).reshape(N, D)
    w_k = np.asarray(inputs["w_k"], np.float32)
    w_v = np.asarray(inputs["w_v"], np.float32)
    w_q = np.asarray(inputs["w_q"], np.float32)
    w_o = np.asarray(inputs["w_o"], np.float32)
    k_comp = np.asarray(inputs["k_comp"], np.float32)
    v_comp = np.asarray(inputs["v_comp"], np.float32)
    k_dec = np.asarray(inputs["k_dec"], np.float32)
    v_dec = np.asarray(inputs["v_dec"], np.float32)

    w_k_eff = w_k * lw_kv[:, None, :]
    w_v_eff = w_v * lw_kv[:, None, :]
    bias_k = np.einsum("ni,noi->no", lb_kv, w_k)
    bias_v = np.einsum("ni,noi->no", lb_kv, w_v)

    w_kc = np.einsum("nro,noi->nri", k_comp, w_k_eff) * mask_f[:, None, None]
    w_vc = np.einsum("nro,noi->nri", v_comp, w_v_eff) * mask_f[:, None, None]
    b_kc = np.einsum("no,nro->nr", bias_k, k_comp) * mask_f[:, None]
    b_vc = np.einsum("no,nro->nr", bias_v, v_comp) * mask_f[:, None]

    sc = 1.0 / np.sqrt(np.float32(HD))
    w_q_eff = (w_q * lw_q[:, None, :]) * sc
    b_q = np.einsum("ni,noi->no", lb_q, w_q) * sc

    k_dec_eff = k_dec / n_active
    v_dec_eff = v_dec / n_active

    # fold k_dec into the q projection: q_dec = normed @ w_qk^T per head,
    # where w_qk[n,h] = k_dec_eff[h-slice].T @ w_q_eff[n, h-slice]  [R, D]
    w_qk = np.stack([
        np.stack([k_dec_eff[h * HD:(h + 1) * HD, :].T
                  @ w_q_eff[n, h * HD:(h + 1) * HD, :] for h in range(H)])
        for n in range(N)])                      # [N, H, R, D]
    b_qk = np.stack([
        np.stack([k_dec_eff[h * HD:(h + 1) * HD, :].T
                  @ b_q[n, h * HD:(h + 1) * HD] for h in range(H)])
        for n in range(N)])                      # [N, H, R]

    # fold v_dec into w_o: wvo[n,h] = (W_oh @ Vd_h)^T  [R, D]; device
    # layout stacks head pairs on partitions: [2R, 2*D] with pair p in
    # columns [p*D,(p+1)*D), even head rows 0:R, odd head rows R:2R
    wvo = np.stack([
        np.stack([(w_o[n][:, h * HD:(h + 1) * HD]
                   @ v_dec_eff[h * HD:(h + 1) * HD, :]).T for h in range(H)])
        for n in range(N)])                      # [N, H, R, D]

    with_kv_bias = bool(np.any(b_kc != 0) or np.any(b_vc != 0))
    with_q_bias = bool(np.any(b_qk != 0))

    ident = np.eye(128, dtype=bf)
    # transposed-causal 0/1 mask for attn^T diag blocks [k, q]:
    # valid (1) where q >= k, 0 strictly below the diagonal
    cmask = np.triu(np.ones((128, 128), np.float32)).astype(bf)

    in_maps = []
    for n in range(N):
        m = {
            "x": x[n].reshape(TOK, D),
            "wkv": np.ascontiguousarray(
                np.concatenate([w_kc[n].T, w_vc[n].T], axis=1)).astype(bf),
            "wqk": np.ascontiguousarray(
                np.concatenate([w_qk[n, h].T for h in range(H)],
                               axis=1)).astype(bf),
            "wvo": np.ascontiguousarray(np.concatenate(
                [np.concatenate([wvo[n, 2 * p], wvo[n, 2 * p + 1]], axis=0)
                 for p in range(2)], axis=1)).astype(bf),
            "ident": ident,
            "cmask": cmask,
        }
        if with_kv_bias:
            m["bkv"] = np.concatenate([b_kc[n], b_vc[n]])[None, :].astype(bf)
        if with_q_bias:
            m["bq"] = b_qk[n].reshape(1, H * R).astype(bf)
        in_maps.append(m)
    return in_maps, with_kv_bias, with_q_bias


def _run(inputs, trace=False):
    from concourse import bass_utils

    in_maps, with_kv_bias, with_q_bias = _prepare(inputs)
    key = (with_kv_bias, with_q_bias)
    if key not in _STATE:
        _STATE[key] = _build_program(with_kv_bias, with_q_bias)
    nc = _STATE[key]
    res = bass_utils.run_bass_kernel_spmd(
        nc, in_maps, core_ids=list(range(N)), trace=trace
    )
    outs = np.stack([np.asarray(res.results[c]["out"]) for c in range(N)])
    out = outs.reshape(N, B, T, D)
    mask_b = np.asarray(inputs["col_mask"]).reshape(N, 1, 1, 1)
    out = np.where(mask_b, out,
                   np.asarray(inputs["col_states"], np.float32))
    return out, res


def kernel(**inputs):
    out, _ = _run(inputs, trace=False)
    return out


# revision 28
# speedup vs baseline: 1.2044x; 1.2044x over previous
"""Trainium2 Bass kernel for BatchedCrossColumnAttentionCompressed.

Strategy (sharding_hint): shard leading N (column) axis across the 8 cores.
Each core: LN -> (folded) compress projections -> quantize -> AllReduce of the
small [TOK, 2R] compressed tensor -> decompress -> causal SDPA -> out proj.

Host-side algebraic folding (exact linear-map collapses):
  - LN affine (w,b) folded into projection weights (biases are zero for the
    actual inputs -> bias paths elided at build time).
  - k/v D->D projection collapsed into the D->R compression: w_kc = k_comp @ w_k_eff.
  - col_mask folded into w_kc/w_vc; 1/n_active folded into decompress weights.
  - 1/sqrt(HD) folded into q projection; k_dec folded into q projection.
  - v_dec folded into w_o: wvo_h = (W_oh @ Vd_h)^T  [R, D] -- the attention
    output projection consumes the compressed context directly (no decompress
    stage on device).
Softmax: scores are tiny (|s| << 1), so max-subtraction is skipped (exact same
math as reference up to fp rounding). exp computed on ACT over 1024-wide PSUM
windows; normalization deferred to phase F (per-head scaling pre-sum).
quant_ste round() implemented with the fp32 magic-constant RNE trick.

Scores matmuls are K=64 row-tiles: kavg^T is duplicated into partitions
64-127 and the two heads of a pair run concurrently on PE row-groups
(0,0)/(64,0). q projection emits head pairs stacked [128, tok] directly.
"""

import numpy as np
import ml_dtypes

N, B, T, D = 8, 4, 1024, 512
H = 4
HD = D // H           # 128
R = 64
R2 = 2 * R            # 128
EPS = 1e-5
TOK = B * T           # 4096
NTI = TOK // 128      # 32 token chunks
KD = D // 128         # 4 contraction chunks
NQ = T // 128         # 8 q-chunks per batch row
MAGIC = 12582912.0    # 1.5 * 2^23 -> round-to-nearest-even trick

_STATE = {}

# attn^T strip packing: strip ki holds q in [ki*128, T). Strips are
# reordered into 1024-wide windows ([s0],[s1,s7],[s2,s6],[s3,s5],[s4]) so
# every 512-aligned exp window is filled and chunks never split banks.
STRIPW = [T - 128 * ki for ki in range(NQ)]
_order = [0, 1, 7, 2, 6, 3, 5, 4]
OFFS = [0] * NQ
_off = 0
for _ki in _order:
    OFFS[_ki] = _off
    _off += STRIPW[_ki]
TOTW = sum(STRIPW)                    # 4608
WINW = 1024                            # exp window width (2 PSUM banks)
NWIN = (TOTW + WINW - 1) // WINW       # 5

# per-window list of (ki, g0, g1): global attnT cols [g0,g1) from strip ki,
# split at 512 (PSUM bank) boundaries
WIN_CHUNKS = []
for w in range(NWIN):
    lo, hi = w * WINW, min((w + 1) * WINW, TOTW)
    chunks = []
    for ki in range(NQ):
        s0, s1 = OFFS[ki], OFFS[ki] + STRIPW[ki]
        c0, c1 = max(s0, lo), min(s1, hi)
        g = c0
        while g < c1:
            nxt = min(c1, (g // 512 + 1) * 512)
            chunks.append((ki, g, nxt))
            g = nxt
    WIN_CHUNKS.append((lo, hi, chunks))


def _build_program(with_kv_bias, with_q_bias):
    from concourse import bacc
    import concourse.bass as bass
    import concourse.tile as tile
    import concourse.mybir as mybir

    f32 = mybir.dt.float32
    bf16 = mybir.dt.bfloat16
    AF = mybir.ActivationFunctionType
    ALU = mybir.AluOpType
    AX = mybir.AxisListType

    nc = bacc.Bacc("TRN2", target_bir_lowering=False, debug=False, num_devices=N)

    x_d = nc.dram_tensor("x", [TOK, D], f32, kind="ExternalInput").ap()
    wkv_d = nc.dram_tensor("wkv", [D, R2], bf16, kind="ExternalInput").ap()
    wqk_d = nc.dram_tensor("wqk", [D, H * R], bf16, kind="ExternalInput").ap()
    wvo_d = nc.dram_tensor("wvo", [R2, 2 * D], bf16, kind="ExternalInput").ap()
    ident_d = nc.dram_tensor("ident", [128, 128], bf16, kind="ExternalInput").ap()
    cmask_d = nc.dram_tensor("cmask", [128, 128], bf16,
                             kind="ExternalInput").ap()
    if with_kv_bias:
        bkv_d = nc.dram_tensor("bkv", [1, R2], bf16, kind="ExternalInput").ap()
    if with_q_bias:
        bq_d = nc.dram_tensor("bq", [1, H * R], bf16, kind="ExternalInput").ap()
    out_d = nc.dram_tensor("out", [TOK, D], f32, kind="ExternalOutput").ap()

    with tile.TileContext(nc) as tc:
        with (
            tc.tile_pool(name="consts", bufs=1) as consts,
            tc.tile_pool(name="big", bufs=1) as big,
            tc.tile_pool(name="work", bufs=3) as work,
            tc.tile_pool(name="work2", bufs=2) as work2,
            tc.tile_pool(name="small", bufs=24) as small,
            # PSUM: psS 2x4KB (scores windows) + psAv 2x2KB + psF 2x2KB = 16KB
            tc.tile_pool(name="psS", bufs=2, space="PSUM") as psS,
            tc.tile_pool(name="psAv", bufs=2, space="PSUM") as psAv,
            tc.tile_pool(name="psF", bufs=2, space="PSUM") as psF,
            tc.tile_pool(name="dram", bufs=1, space="DRAM") as dpool,
        ):
            # AR chunk sizes in 128-token tiles: first batch row split in
            # half so the first collective (whose staging is BW-bound) lands
            # as early as possible
            SPLITS = [8, 8, 8, 8]
            CUM = [0]
            for s in SPLITS:
                CUM.append(CUM[-1] + s)
            ar_ins = [dpool.tile([SPLITS[q] * 128, R2], bf16,
                                 name=f"ar_in{q}") for q in range(len(SPLITS))]
            ar_outs = [dpool.tile([SPLITS[q] * 128, R2], bf16,
                                  name=f"ar_out{q}", addr_space="Shared")
                       for q in range(len(SPLITS))]

            def ar_buf(ti):
                q = 0
                while CUM[q + 1] <= ti:
                    q += 1
                return q, ti - CUM[q]

            # ---- constants ----
            ident = consts.tile([128, 128], bf16, name="ident")
            nc.sync.dma_start(out=ident, in_=ident_d)
            cmask = consts.tile([128, 128], bf16, name="cmask")
            nc.sync.dma_start(out=cmask, in_=cmask_d)
            wkv_s = []
            for kd in range(KD):
                wkvt = consts.tile([128, R2], bf16, name=f"wkv{kd}")
                nc.sync.dma_start(out=wkvt, in_=wkv_d[kd * 128:(kd + 1) * 128, :])
                wkv_s.append(wkvt)
            wqk_s = []
            for kd in range(KD):
                wqkt = consts.tile([128, H * R], bf16, name=f"wqk{kd}")
                nc.sync.dma_start(out=wqkt,
                                  in_=wqk_d[kd * 128:(kd + 1) * 128, :])
                wqk_s.append(wqkt)
            wvo_sb = consts.tile([R2, 2 * D], bf16, name="wvo_sb")
            nc.sync.dma_start(out=wvo_sb, in_=wvo_d)
            ones_col = consts.tile([128, 1], bf16, name="ones_col")
            nc.vector.memset(ones_col, 1.0)
            eps_t = consts.tile([128, 1], f32, name="eps_t")
            nc.vector.memset(eps_t, EPS)
            ones_row = consts.tile([1, 512], bf16, name="ones_row")
            nc.vector.memset(ones_row, 1.0)
            if with_kv_bias:
                bkv_s = consts.tile([1, R2], bf16, name="bkv_s")
                nc.sync.dma_start(out=bkv_s, in_=bkv_d)
            if with_q_bias:
                bq_s = consts.tile([1, H * R], bf16, name="bq_s")
                nc.sync.dma_start(out=bq_s, in_=bq_d)

            # ---- persistent big tensors ----
            nt = big.tile([128, KD, TOK], bf16, name="nt")
            # q projected into compressed space, head-PAIRS stacked on
            # partitions: rows 0-63 = head 2p, 64-127 = head 2p+1
            qdecT = big.tile([128, 2, TOK], bf16, name="qdecT")
            # kavg^T duplicated into both partition halves (row-tiling)
            kavgT = big.tile([128, TOK], bf16, name="kavgT")
            # v_avg natural chunks with a ones column at index R (Z trick)
            vext = big.tile([128, NTI, R + 1], bf16, name="vext")
            # compressed context per batch (double-buffered on b parity):
            # rows 0-63 ctx, row 64 = Z
            outcB = big.tile([R + 1, H, 2, T], bf16, name="outcB")
            # odd heads' ctx shifted to partitions 64-127 so phase-F matmuls
            # alternate PE row-halves (complementary row-groups overlap)
            outcHi = big.tile([128, 2, 2, T], bf16, name="outcHi")
            recipsAll = big.tile([128, B * H * NQ], f32, name="recipsAll")

            # ================= Phase A: LN + transpose + compress + quant ====
            def emit_A(t0, t1):
                for ti in range(t0, t1):
                    tsl = slice(ti * 128, (ti + 1) * 128)
                    xt = work.tile([128, D], f32, tag="xt", bufs=8, name="xt")
                    nc.sync.dma_start(out=xt, in_=x_d[tsl, :])
                    stats = small.tile([128, 6], f32, name="stats")
                    nc.vector.bn_stats(out=stats, in_=xt)
                    mv = small.tile([128, 2], f32, name="mv")
                    nc.vector.bn_aggr(out=mv, in_=stats)
                    std = small.tile([128, 1], f32, name="std")
                    nc.scalar.activation(out=std, in_=mv[:, 1:2], func=AF.Sqrt,
                                         bias=eps_t, scale=1.0)
                    rstd = small.tile([128, 1], f32, name="rstd")
                    nc.vector.reciprocal(out=rstd, in_=std)
                    # nbias = -mean*rstd ; normed = x*rstd + nbias  (on ACT)
                    nbias = small.tile([128, 1], f32, name="nbias")
                    nc.vector.tensor_scalar(out=nbias, in0=mv[:, 0:1],
                                            scalar1=rstd, scalar2=-1.0,
                                            op0=ALU.mult, op1=ALU.mult)
                    nrm = work.tile([128, D], bf16, tag="nrm", bufs=8, name="nrm")
                    nc.scalar.activation(out=nrm, in_=xt, func=AF.Identity,
                                         bias=nbias, scale=rstd)
                    pst = psS.tile([128, KD * 128], bf16, tag="psS", name="pst")
                    for kd in range(KD):
                        nc.tensor.transpose(pst[:, kd * 128:(kd + 1) * 128],
                                            nrm[:, kd * 128:(kd + 1) * 128],
                                            ident)
                    nc.vector.tensor_copy(
                        out=nt[:, :, tsl],
                        in_=pst.rearrange("p (g c) -> p g c", g=KD))
                    pskv = psAv.tile([128, R2], f32, tag="psAv", name="pskv")
                    for kd in range(KD):
                        nc.tensor.matmul(pskv, lhsT=nt[:, kd, tsl], rhs=wkv_s[kd],
                                         start=(kd == 0),
                                         stop=(kd == KD - 1 and not with_kv_bias))
                    if with_kv_bias:
                        nc.tensor.matmul(pskv, lhsT=bkv_s, rhs=ones_row[:, 0:128],
                                         start=False, stop=True)
                    absm = small.tile([128, 2], f32, name="absm")
                    nc.vector.tensor_reduce(
                        out=absm,
                        in_=pskv.rearrange("p (g r) -> p g r", g=2),
                        axis=AX.X, op=ALU.max, apply_absolute_value=True)
                    # inv_s = max(absm,1e-8)/127 ; sc = 1/inv_s ; mb = -MAGIC*inv_s
                    inv_s = small.tile([128, 2], f32, name="inv_s")
                    nc.vector.tensor_scalar(out=inv_s, in0=absm, scalar1=1e-8,
                                            scalar2=1.0 / 127.0, op0=ALU.max,
                                            op1=ALU.mult)
                    sc = small.tile([128, 2], f32, name="sc")
                    nc.vector.reciprocal(out=sc, in_=inv_s)
                    mb = small.tile([128, 2], f32, name="mb")
                    nc.vector.tensor_scalar_mul(out=mb, in0=inv_s, scalar1=-MAGIC)
                    arq = work.tile([128, R2], bf16, tag="arq", bufs=8, name="arq")
                    tmpq = work.tile([128, R2], f32, tag="tmpq", bufs=8, name="tmpq")
                    for half in range(2):
                        sl = slice(half * R, (half + 1) * R)
                        hh = slice(half, half + 1)
                        # y = x*sc + MAGIC  (rounds to int in fp32 mantissa)
                        if half == 0:
                            nc.vector.tensor_scalar(out=tmpq[:, sl],
                                                    in0=pskv[:, sl],
                                                    scalar1=sc[:, hh],
                                                    scalar2=MAGIC,
                                                    op0=ALU.mult, op1=ALU.add)
                        else:
                            nc.scalar.activation(out=tmpq[:, sl],
                                                 in_=pskv[:, sl],
                                                 func=AF.Copy, bias=MAGIC,
                                                 scale=sc[:, hh])
                        # q = (y - MAGIC)*inv_s = y*inv_s + mb
                        nc.scalar.activation(out=arq[:, sl], in_=tmpq[:, sl],
                                             func=AF.Identity, bias=mb[:, hh],
                                             scale=inv_s[:, hh])
                    q_, r_ = ar_buf(ti)
                    nc.sync.dma_start(
                        out=ar_ins[q_][r_ * 128:(r_ + 1) * 128, :],
                        in_=arq)

            # ================= Phase B: AllReduce (quartered pipeline) ======
            def emit_AR(which):
                nc.gpsimd.collective_compute(
                    "AllReduce",
                    ALU.add,
                    replica_groups=[list(range(N))],
                    ins=[ar_ins[which].opt()],
                    outs=[ar_outs[which].opt()],
                )

            # ================= Phase C: q^T pair projection (overlaps AR) ===
            def emit_C(n0, n1):
                for p in range(2):
                    for nch in range(n0, n1):
                        csl = slice(nch * 512, (nch + 1) * 512)
                        psq = psF.tile([128, 512], f32, tag="psF", name="psq")
                        for kd in range(KD):
                            nc.tensor.matmul(
                                psq,
                                lhsT=wqk_s[kd][:, p * 128:(p + 1) * 128],
                                rhs=nt[:, kd, csl],
                                start=(kd == 0),
                                stop=(kd == KD - 1 and not with_q_bias),
                            )
                        if with_q_bias:
                            nc.tensor.matmul(psq,
                                             lhsT=bq_s[:, p * 128:(p + 1) * 128],
                                             rhs=ones_row, start=False, stop=True)
                        nc.vector.tensor_copy(out=qdecT[:, p, csl], in_=psq)

            # ================= Phase D: k_avg^T dup transpose + v_ext =======
            def emit_D(t0, t1):
                # one strided DMA per 8-tile quarter: out[p, r, c] takes row
                # r*128+p of the AR result (saves 7 queue issues per quarter
                # right on the post-collective ramp)
                nch = t1 - t0
                avgN = work.tile([128, nch, R2], bf16, tag="avgN", bufs=2,
                                 name="avgN")
                q0_, r0_ = ar_buf(t0)
                nc.sync.dma_start(
                    out=avgN,
                    in_=ar_outs[q0_][r0_ * 128:(r0_ + nch) * 128, :]
                    .rearrange("(r p) c -> p r c", p=128))
                for ti in range(t0, t1):
                    tsl = slice(ti * 128, (ti + 1) * 128)
                    av = avgN[:, ti - t0, :]
                    psK = psAv.tile([128, 128], f32, tag="psAv", name="psK")
                    nc.tensor.matmul(psK[0:64, :], lhsT=av[:, 0:R], rhs=ident,
                                     start=True, stop=True)
                    nc.tensor.matmul(psK[64:128, :], lhsT=av[:, 0:R], rhs=ident,
                                     start=True, stop=True)
                    nc.vector.tensor_copy(out=kavgT[:, tsl], in_=psK)
                    nc.gpsimd.tensor_copy(out=vext[:, ti, 0:R],
                                          in_=av[:, R:R2])

            # ================= Phase E: causal SDPA (compressed, rank-R) ====
            # scoresT strips for the two heads of a pair computed CONCURRENTLY
            # as K=64 row-tiles (kavgT dup at partitions 64-127). exp over
            # 1024-wide PSUM windows -> attn^T strips in SBUF. AV contracts
            # attn^T against [v_avg | ones] chunks: psc row R gives Z, rows
            # 0..R-1 the compressed context (-> outcB, consumed by phase F).
            GQ = 4  # q-chunks per AV group (512-wide matmuls)
            def emit_E(b, p):
                base = b * T
                if True:
                    attnP = work2.tile([128, 2, TOTW], bf16, name="attnP")
                    attnE = attnP[:, 0, :]
                    attnO = attnP[:, 1, :]
                    for w in range(NWIN):
                        lo, hi, chunks = WIN_CHUNKS[w]
                        tE = psS.tile([128, WINW], f32, tag="psS", name="tE")
                        tO = psS.tile([128, WINW], f32, tag="psS", name="tO")
                        for (ki, g0, g1) in chunks:
                            q0 = base + ki * 128 + (g0 - OFFS[ki])
                            q1 = q0 + (g1 - g0)
                            kb = slice(base + ki * 128, base + (ki + 1) * 128)
                            nc.tensor.matmul(
                                tE[:, g0 - lo:g1 - lo],
                                lhsT=kavgT[0:64, kb],
                                rhs=qdecT[0:64, p, q0:q1],
                                start=True, stop=True)
                            nc.tensor.matmul(
                                tO[:, g0 - lo:g1 - lo],
                                lhsT=kavgT[64:128, kb],
                                rhs=qdecT[64:128, p, q0:q1],
                                start=True, stop=True)
                        nc.scalar.activation(out=attnE[:, lo:hi],
                                             in_=tE[:, 0:hi - lo], func=AF.Exp)
                        nc.scalar.activation(out=attnO[:, lo:hi],
                                             in_=tO[:, 0:hi - lo], func=AF.Exp)
                        # causal zeroing of diag blocks living in this window
                        for ki in range(NQ):
                            if lo <= OFFS[ki] < hi:
                                dsl = slice(OFFS[ki], OFFS[ki] + 128)
                                nc.vector.tensor_tensor(
                                    out=attnE[:, dsl], in0=attnE[:, dsl],
                                    in1=cmask, op=ALU.mult)
                                nc.vector.tensor_tensor(
                                    out=attnO[:, dsl], in0=attnO[:, dsl],
                                    in1=cmask, op=ALU.mult)
                    for h, attnTs in ((2 * p, attnE), (2 * p + 1, attnO)):
                        for g in range(NQ // GQ):
                            q0 = g * GQ          # first q-chunk of group
                            gw = GQ * 128        # 512
                            gsl = slice(q0 * 128, (q0 + GQ) * 128)
                            psc = psAv.tile([R + 1, gw], f32, tag="psAv", name="psc")
                            for ki in range(q0 + GQ):
                                lo2 = max(ki, q0)
                                nc.tensor.matmul(
                                    psc[:, (lo2 - q0) * 128:gw],
                                    lhsT=vext[:, b * NQ + ki, :],
                                    rhs=attnTs[:, OFFS[ki] + (lo2 - ki) * 128:
                                               OFFS[ki] +
                                               (q0 + GQ - ki) * 128],
                                    start=(ki == 0), stop=(ki == q0 + GQ - 1),
                                    skip_group_check=True)
                            if g % 2 == 0:
                                nc.vector.tensor_copy(
                                    out=outcB[:, h, b % 2, gsl], in_=psc)
                            else:
                                nc.scalar.copy(
                                    out=outcB[:, h, b % 2, gsl], in_=psc)
                        # Z row -> psz columns (outer-product transposes)
                        psz = psAv.tile([128, NQ], f32, tag="psAv", name="psz")
                        for qi in range(NQ):
                            nc.tensor.matmul(
                                psz[:, qi:qi + 1],
                                lhsT=outcB[R:R + 1, h, b % 2,
                                           qi * 128:(qi + 1) * 128],
                                rhs=ones_col[R:R + 1, 0:1],
                                start=True, stop=True)
                        idx0 = (b * H + h) * NQ
                        zcol = small.tile([128, NQ], f32, name="zcol")
                        nc.vector.tensor_copy(out=zcol, in_=psz)
                        nc.vector.reciprocal(
                            out=recipsAll[:, idx0:idx0 + NQ], in_=zcol)
                        if h % 2 == 1:
                            nc.sync.dma_start(
                                out=outcHi[64:128, h // 2, b % 2, :],
                                in_=outcB[0:R, h, b % 2, :])
            # ================= Phase F: out proj + residual + normalize =====
            # of[tok, D] = x + sum_h (1/Z_h) * (outc_h^T @ wvo_h)   (K = R)
            # Emitted in two passes (one per head pair) so pass 0 of batch b
            # overlaps the second E pair of the same batch.
            ofs = {}

            def emit_F(b, p):
                for qi in range(NQ):
                    ti = b * NQ + qi
                    tsl = slice(ti * 128, (ti + 1) * 128)
                    qsl = slice(qi * 128, (qi + 1) * 128)
                    if p == 0:
                        xt2 = work.tile([128, D], f32, tag="xt", bufs=8,
                                        name="xt")
                        nc.sync.dma_start(out=xt2, in_=x_d[tsl, :])
                        of = work.tile([128, D], f32, tag="of", bufs=10,
                                       name="of")
                        ofs[qi] = (of, xt2)
                    else:
                        of, xt2 = ofs[qi]
                    psoE = psF.tile([128, 512], f32, tag="psF", name="psoE")
                    psoO = psF.tile([128, 512], f32, tag="psF", name="psoO")
                    nc.tensor.matmul(
                        psoE,
                        lhsT=outcB[0:R, 2 * p, b % 2, qsl],
                        rhs=wvo_sb[0:R, p * D:(p + 1) * D],
                        start=True, stop=True)
                    nc.tensor.matmul(
                        psoO,
                        lhsT=outcHi[64:128, p, b % 2, qsl],
                        rhs=wvo_sb[64:128, p * D:(p + 1) * D],
                        start=True, stop=True)
                    r0 = (b * H + 2 * p) * NQ + qi
                    r1 = (b * H + 2 * p + 1) * NQ + qi
                    nc.vector.scalar_tensor_tensor(
                        out=of, in0=psoE,
                        scalar=recipsAll[:, r0:r0 + 1],
                        in1=(xt2 if p == 0 else of),
                        op0=ALU.mult, op1=ALU.add)
                    nc.vector.scalar_tensor_tensor(
                        out=of, in0=psoO,
                        scalar=recipsAll[:, r1:r1 + 1],
                        in1=of, op0=ALU.mult, op1=ALU.add)
                    if p == 1:
                        nc.sync.dma_start(out=out_d[tsl, :], in_=of)

            # ---- pipelined emission order ----
            nc.vector.memset(vext[:, :, R:R + 1], 1.0)
            with nc.named_scope("A1"):
                emit_A(0, 8)
            with nc.named_scope("AR0"):
                emit_AR(0)
            with nc.named_scope("C1"):
                emit_C(0, 2)
            with nc.named_scope("A2"):
                emit_A(8, 16)
            with nc.named_scope("AR1"):
                emit_AR(1)
            with nc.named_scope("C2"):
                emit_C(2, 4)
            with nc.named_scope("A3"):
                emit_A(16, 24)
            with nc.named_scope("AR2"):
                emit_AR(2)
            with nc.named_scope("C3"):
                emit_C(4, 6)
            with nc.named_scope("A4"):
                emit_A(24, 32)
            with nc.named_scope("AR3"):
                emit_AR(3)
            with nc.named_scope("C4"):
                emit_C(6, 8)
            with nc.named_scope("D1"):
                emit_D(0, 8)
            with nc.named_scope("E0a"):
                emit_E(0, 0)
            with nc.named_scope("E0b"):
                emit_E(0, 1)
            with nc.named_scope("F0a"):
                emit_F(0, 0)
            with nc.named_scope("D2"):
                emit_D(8, 16)
            with nc.named_scope("F0b"):
                emit_F(0, 1)
            with nc.named_scope("E1a"):
                emit_E(1, 0)
            with nc.named_scope("F1a"):
                emit_F(1, 0)
            with nc.named_scope("E1b"):
                emit_E(1, 1)
            with nc.named_scope("D3"):
                emit_D(16, 24)
            with nc.named_scope("F1b"):
                emit_F(1, 1)
            with nc.named_scope("E2a"):
                emit_E(2, 0)
            with nc.named_scope("F2a"):
                emit_F(2, 0)
            with nc.named_scope("E2b"):
                emit_E(2, 1)
            with nc.named_scope("D4"):
                emit_D(24, 32)
            with nc.named_scope("F2b"):
                emit_F(2, 1)
            with nc.named_scope("E3a"):
                emit_E(3, 0)
            with nc.named_scope("F3a"):
                emit_F(3, 0)
            with nc.named_scope("E3b"):
                emit_E(3, 1)
            with nc.named_scope("F3b"):
                emit_F(3, 1)

    nc.compile()
    return nc


def _prepare(inputs):
    bf = ml_dtypes.bfloat16
    x = np.ascontiguousarray(np.asarray(inputs["col_states"], np.float32))
    mask_f = np.asarray(inputs["col_mask"]).astype(np.float32)
    n_active = max(float(mask_f.sum()), 1.0)

    lw_kv = np.asarray(inputs["ln_kv_w"], np.float32).reshape(N, D)
    lb_kv = np.asarray(inputs["ln_kv_b"], np.float32).reshape(N, D)
    lw_q = np.asarray(inputs["ln_q_w"], np.float32).reshape(N, D)
    lb_q = np.asarray(inputs["ln_q_b"], np.float32).reshape(N, D)
    w_k = np.asarray(inputs["w_k"], np.float32)
    w_v = np.asarray(inputs["w_v"], np.float32)
    w_q = np.asarray(inputs["w_q"], np.float32)
    w_o = np.asarray(inputs["w_o"], np.float32)
    k_comp = np.asarray(inputs["k_comp"], np.float32)
    v_comp = np.asarray(inputs["v_comp"], np.float32)
    k_dec = np.asarray(inputs["k_dec"], np.float32)
    v_dec = np.asarray(inputs["v_dec"], np.float32)

    w_k_eff = w_k * lw_kv[:, None, :]
    w_v_eff = w_v * lw_kv[:, None, :]
    bias_k = np.einsum("ni,noi->no", lb_kv, w_k)
    bias_v = np.einsum("ni,noi->no", lb_kv, w_v)

    w_kc = np.einsum("nro,noi->nri", k_comp, w_k_eff) * mask_f[:, None, None]
    w_vc = np.einsum("nro,noi->nri", v_comp, w_v_eff) * mask_f[:, None, None]
    b_kc = np.einsum("no,nro->nr", bias_k, k_comp) * mask_f[:, None]
    b_vc = np.einsum("no,nro->nr", bias_v, v_comp) * mask_f[:, None]

    sc = 1.0 / np.sqrt(np.float32(HD))
    w_q_eff = (w_q * lw_q[:, None, :]) * sc
    b_q = np.einsum("ni,noi->no", lb_q, w_q) * sc

    k_dec_eff = k_dec / n_active
    v_dec_eff = v_dec / n_active

    # fold k_dec into the q projection: q_dec = normed @ w_qk^T per head,
    # where w_qk[n,h] = k_dec_eff[h-slice].T @ w_q_eff[n, h-slice]  [R, D]
    w_qk = np.stack([
        np.stack([k_dec_eff[h * HD:(h + 1) * HD, :].T
                  @ w_q_eff[n, h * HD:(h + 1) * HD, :] for h in range(H)])
        for n in range(N)])                      # [N, H, R, D]
    b_qk = np.stack([
        np.stack([k_dec_eff[h * HD:(h + 1) * HD, :].T
                  @ b_q[n, h * HD:(h + 1) * HD] for h in range(H)])
        for n in range(N)])                      # [N, H, R]

    # fold v_dec into w_o: wvo[n,h] = (W_oh @ Vd_h)^T  [R, D]; device
    # layout stacks head pairs on partitions: [2R, 2*D] with pair p in
    # columns [p*D,(p+1)*D), even head rows 0:R, odd head rows R:2R
    wvo = np.stack([
        np.stack([(w_o[n][:, h * HD:(h + 1) * HD]
                   @ v_dec_eff[h * HD:(h + 1) * HD, :]).T for h in range(H)])
        for n in range(N)])                      # [N, H, R, D]

    with_kv_bias = bool(np.any(b_kc != 0) or np.any(b_vc != 0))
    with_q_bias = bool(np.any(b_qk != 0))

    ident = np.eye(128, dtype=bf)
    # transposed-causal 0/1 mask for attn^T diag blocks [k, q]:
    # valid (1) where q >= k, 0 strictly below the diagonal
    cmask = np.triu(np.ones((128, 128), np.float32)).astype(bf)

    in_maps = []
    for n in range(N):
        m = {
            "x": x[n].reshape(TOK, D),
            "wkv": np.ascontiguousarray(
                np.concatenate([w_kc[n].T, w_vc[n].T], axis=1)).astype(bf),
            "wqk": np.ascontiguousarray(
                np.concatenate([w_qk[n, h].T for h in range(H)],
                               axis=1)).astype(bf),
            "wvo": np.ascontiguousarray(np.concatenate(
                [np.concatenate([wvo[n, 2 * p], wvo[n, 2 * p + 1]], axis=0)
                 for p in range(2)], axis=1)).astype(bf),
            "ident": ident,
            "cmask": cmask,
        }
        if with_kv_bias:
            m["bkv"] = np.concatenate([b_kc[n], b_vc[n]])[None, :].astype(bf)
        if with_q_bias:
            m["bq"] = b_qk[n].reshape(1, H * R).astype(bf)
        in_maps.append(m)
    return in_maps, with_kv_bias, with_q_bias


def _run(inputs, trace=False):
    from concourse import bass_utils

    in_maps, with_kv_bias, with_q_bias = _prepare(inputs)
    key = (with_kv_bias, with_q_bias)
    if key not in _STATE:
        _STATE[key] = _build_program(with_kv_bias, with_q_bias)
    nc = _STATE[key]
    res = bass_utils.run_bass_kernel_spmd(
        nc, in_maps, core_ids=list(range(N)), trace=trace
    )
    outs = np.stack([np.asarray(res.results[c]["out"]) for c in range(N)])
    out = outs.reshape(N, B, T, D)
    mask_b = np.asarray(inputs["col_mask"]).reshape(N, 1, 1, 1)
    out = np.where(mask_b, out,
                   np.asarray(inputs["col_states"], np.float32))
    return out, res


def kernel(**inputs):
    out, _ = _run(inputs, trace=False)
    return out
